# revision 16
# baseline (speedup 1.0000x reference)
"""Trainium2 Bass kernel for batched NNLS via accelerated projected gradient.

Strategy (hardcoded for M=1024, K=32, N=16384, 8 cores):
  - Shard the n (column) dimension of X across 8 cores (2048 cols/core).
  - Host precomputes tiny A-only quantities exactly as the fp32 reference
    does where shared (G = A^T A, power iteration for L, step = 1/L,
    H = I - step*G), plus a ridge-regularized warm-start map
    Mw = (G + delta*I)^{-1} A^T and a tuned momentum schedule
    (t-rule ramp for the first RAMP iters, then constant BETA).
  - Device per core, phase 1: B = (step*A)^T X_shard and S0 = relu(Mw X_shard)
    via accumulating fp32 matmuls (k=32 packed 4x along partitions so every
    op runs 128 partitions wide; free dim is the 512 stacked columns).
  - Iteration i = 1..ITERS (same fixed point as the reference's FISTA, so it
    converges to the same solution; the tuned schedule + warm start reach the
    2e-2 gate with ~10x margin in 48 iterations instead of 500):
      ps = H' @ V            (PE, fp32, fixed block-diag weights)
      W  = c_{i-1}*ps + B    (DVE scalar_tensor_tensor, PSUM read)
      R  = relu(W)           (ACT)
      V' = R - g_i*R_prev    (Pool/GPSIMD scalar_tensor_tensor, SBUF only)
    with V = Y/c the momentum iterate rescaled so per-iteration scalars ride
    in instruction immediates.  Four engines pipeline across 2 column halves.
  - Output S = R_final directly (it is already relu(W_final)).
"""

import os
import sys

import numpy as np

sys.path.insert(0, "/opt/trn_rl_repo")

M_DIM, K_DIM, N_DIM = 1024, 32, 16384
N_CORES = 8
N_SHARD = N_DIM // N_CORES          # 2048
N_STACK = N_SHARD // 4              # 512 cols in 4-group stacked layout
ITERS = int(os.environ.get("NNLS_KERNEL_ITERS", "48"))
HALF = N_STACK // 2                 # 256
DELTA = 3.0                         # warm-start ridge
RAMP = 8                            # t-rule momentum ramp length
BETA = 0.86                         # constant momentum after ramp
V_ENGINE = os.environ.get("NNLS_V_ENGINE", "dve")  # dve | pool | split
SCHEME = os.environ.get("NNLS_SCHEME", "shift")    # shift | base
D_ENGINE = os.environ.get("NNLS_D_ENGINE", "dve")   # dve | pool | split
CHUNKS = int(os.environ.get("NNLS_CHUNKS", "2"))    # independent col pipelines


def _host_constants(A: np.ndarray):
    """fp32 replication of the reference's G/L/step plus warm start + schedule."""
    f32 = np.float32
    A = A.astype(f32)
    G = (A.T @ A).astype(f32)
    v = np.ones(K_DIM, dtype=f32) / f32(np.sqrt(f32(K_DIM)))
    for _ in range(100):
        w = (G @ v).astype(f32)
        v = (w / (f32(np.linalg.norm(w)) + f32(1e-30))).astype(f32)
    L = f32(v @ (G @ v))
    step = f32(1.0) / L
    H = (np.eye(K_DIM, dtype=f32) - step * G).astype(f32)

    Mw = np.linalg.solve(
        (G + f32(DELTA) * np.eye(K_DIM, dtype=f32)).astype(np.float64),
        A.T.astype(np.float64),
    ).astype(f32)                            # (k, m) warm-start map
    Minv = np.linalg.solve(
        G.astype(np.float64), A.T.astype(np.float64)
    ).astype(f32)                            # (k, m): W_fix = Minv @ X

    t = 1.0
    mus = []
    for j in range(1024):
        tn = 0.5 * (1.0 + np.sqrt(1.0 + 4.0 * t * t))
        mu = (t - 1.0) / tn
        t = tn
        mus.append(min(mu, BETA) if j < RAMP else BETA)
    mus = np.array(mus, dtype=f32)           # mus[j] = mu_{j+1}
    cs = (1.0 + mus).astype(f32)             # cs[j] = 1 + mu_{j+1}
    gs = (mus / (1.0 + mus)).astype(f32)     # gs[j] = g_{j+1}
    return step, H, Mw, Minv, cs, gs


def _build_program(iters: int):
    import concourse.bass as bass
    import concourse.tile as tile
    from concourse import bacc, mybir
    from contextlib import ExitStack

    f32 = mybir.dt.float32
    AL = mybir.AluOpType
    nc = bacc.Bacc("TRN2", target_bir_lowering=False, debug=False,
                   num_devices=N_CORES)

    Hd_d = nc.dram_tensor("Hd", [128, 128], f32, kind="ExternalInput").ap()
    if SCHEME == "base":
        Apk_d = nc.dram_tensor("Apk", [128, 8 * K_DIM], f32,
                               kind="ExternalInput").ap()
    else:
        Npk_d = nc.dram_tensor("Npk", [128, 8 * K_DIM], f32,
                               kind="ExternalInput").ap()
    Mpk_d = nc.dram_tensor("Mpk", [128, 8 * K_DIM], f32, kind="ExternalInput").ap()
    Xs_d = nc.dram_tensor("Xs", [8, 128, N_SHARD], f32, kind="ExternalInput").ap()
    Sout_d = nc.dram_tensor("Sout", [128, N_STACK], f32, kind="ExternalOutput").ap()

    cs_arr, gs_arr = _build_program.consts
    Relu = mybir.ActivationFunctionType.Relu
    Copy = mybir.ActivationFunctionType.Copy

    with tile.TileContext(nc) as tc, ExitStack() as ctx:
        const_pool = ctx.enter_context(tc.tile_pool(name="const", bufs=1))
        xpool = ctx.enter_context(tc.tile_pool(name="x", bufs=1))
        bpool = ctx.enter_context(tc.tile_pool(name="b", bufs=1))
        wpool = ctx.enter_context(tc.tile_pool(name="w", bufs=3))
        rpool = ctx.enter_context(tc.tile_pool(name="r", bufs=3))
        vpool = ctx.enter_context(tc.tile_pool(name="v", bufs=2))
        psA_pool = ctx.enter_context(tc.tile_pool(name="psA", bufs=1, space="PSUM"))
        psW_pool = ctx.enter_context(tc.tile_pool(name="psW", bufs=1, space="PSUM"))
        ps_pool = ctx.enter_context(tc.tile_pool(name="ps", bufs=2, space="PSUM"))
        scr_pool = ctx.enter_context(tc.tile_pool(name="scr", bufs=1, space="PSUM"))

        Hd = const_pool.tile([128, 128], f32, tag="Hd")
        nc.sync.dma_start(out=Hd[:], in_=Hd_d[:])
        if SCHEME == "base":
            Apk = const_pool.tile([128, 8 * K_DIM], f32, tag="Apk")
            nc.sync.dma_start(out=Apk[:], in_=Apk_d[:])
        else:
            Npk = const_pool.tile([128, 8 * K_DIM], f32, tag="Npk")
            nc.sync.dma_start(out=Npk[:], in_=Npk_d[:])
        Mpk = const_pool.tile([128, 8 * K_DIM], f32, tag="Mpk")
        nc.sync.dma_start(out=Mpk[:], in_=Mpk_d[:])

        # LDW-bearing matmuls only support one sync-wait command; these dummy
        # matmuls fold the const DMA ticks into PE's vector clock so every
        # real matmul below needs at most one wait.
        consts2 = (Apk,) if SCHEME == "base" else (Npk,)
        for ci, ct in enumerate((Hd, Mpk) + consts2):
            scr = scr_pool.tile([128, 8], f32, tag="scr", name=f"scr{ci}")
            nc.tensor.matmul(scr[0:1, 0:1], ct[:, 0:1], ct[:, 0:1],
                             start=True, stop=True)

        # ---- Phase 1 reductions over X (4 column groups stacked on partitions)
        xts = []
        for j in range(8):
            xt = xpool.tile([128, N_SHARD], f32, tag=f"xt{j}", name=f"xt{j}")
            nc.sync.dma_start(out=xt[:], in_=Xs_d[j])
            xts.append(xt)

        def reduce_chain(weights, pstile):
            for grp in range(4):
                for j in range(8):
                    nc.tensor.matmul(
                        pstile[32 * grp:32 * grp + 32, :],
                        weights[:, 32 * j:32 * j + 32],
                        xts[j][:, N_STACK * grp:N_STACK * (grp + 1)],
                        start=(j == 0),
                        stop=(j == 7),
                        tile_position=(0, 32 * grp),
                    )

        psW = psW_pool.tile([128, N_STACK], f32, tag="psW")
        reduce_chain(Mpk, psW)                   # Mw @ X (ridge warm start)
        S0 = bpool.tile([128, N_STACK], f32, tag="S0")
        nc.scalar.activation(S0[:], psW[:], Relu)

        if SCHEME == "base":
            psB = psA_pool.tile([128, N_STACK], f32, tag="psB")
            reduce_chain(Apk, psB)               # (step*A)^T X
            B = bpool.tile([128, N_STACK], f32, tag="B")
            nc.vector.tensor_scalar_add(B[:], psB[:], 0.0)

            Vh = [S0[:, 0:HALF], S0[:, HALF:N_STACK]]
            Rprev = [S0[:, 0:HALF], S0[:, HALF:N_STACK]]
            for i in range(1, iters + 1):
                c = 1.0 if i == 1 else float(cs_arr[i - 2])   # c_{i-1}
                g = float(gs_arr[i - 1])                      # g_i
                for h in range(2):
                    sl = slice(HALF * h, HALF * (h + 1))
                    ps = ps_pool.tile([128, HALF], f32, tag=f"ps{h}")
                    nc.tensor.matmul(ps[:], Hd[:], Vh[h][:],
                                     start=True, stop=True)
                    W = wpool.tile([128, HALF], f32, tag=f"W{h}")
                    nc.vector.scalar_tensor_tensor(
                        W[:], ps[:], c, B[:, sl], AL.mult, AL.add)
                    R = rpool.tile([128, HALF], f32, tag=f"R{h}")
                    nc.scalar.activation(R[:], W[:], Relu)
                    if i < iters:
                        V = vpool.tile([128, HALF], f32, tag=f"V{h}")
                        veng = (nc.gpsimd if V_ENGINE == "pool"
                                or (V_ENGINE == "split" and h == 0)
                                else nc.vector)
                        veng.scalar_tensor_tensor(
                            V[:], Rprev[h][:], -g, R[:], AL.mult, AL.add)
                        Vh[h] = V
                    Rprev[h] = R
            for h in range(2):
                sl = slice(HALF * h, HALF * (h + 1))
                nc.sync.dma_start(out=Sout_d[:, sl], in_=Rprev[h][:])
        else:
            # Shifted iteration: E = S - Wfix, NF = -Wfix.
            #   ps = H @ D~       (PE)
            #   E' = max(c*ps,NF) (DVE stt, PSUM read)
            #   T' = g_{i+1}*E'   (ACT copy-scale; consumed next iteration)
            #   D~' = E' - T      (Pool tensor_tensor)
            # Output S = E_final - NF.
            psN = psA_pool.tile([128, N_STACK], f32, tag="psN")
            reduce_chain(Npk, psN)               # Wfix = G^-1 A^T X
            NF = bpool.tile([128, N_STACK], f32, tag="NF")
            nc.vector.tensor_scalar_mul(NF[:], psN[:], -1.0)
            E0 = bpool.tile([128, N_STACK], f32, tag="E0")
            nc.vector.tensor_tensor(E0[:], S0[:], NF[:], AL.add)

            CW = N_STACK // CHUNKS               # columns per chunk
            ps_bufs = 2 if CHUNKS <= 2 else 1
            slices = [slice(CW * h, CW * (h + 1)) for h in range(CHUNKS)]
            Dh = [E0[:, sl] for sl in slices]
            Tprev = [None] * CHUNKS
            Eprev = [E0[:, sl] for sl in slices]
            Eh = [None] * CHUNKS
            for i in range(1, iters + 1):
                c = 1.0 if i == 1 else float(cs_arr[i - 2])   # c_{i-1}
                gnext = float(gs_arr[i])                      # g_{i+1}
                g = float(gs_arr[i - 1])                      # g_i
                for h in range(CHUNKS):
                    sl = slices[h]
                    ps = ps_pool.tile([128, CW], f32, tag=f"ps{h}",
                                      bufs=ps_bufs)
                    nc.tensor.matmul(ps[:], Hd[:], Dh[h][:],
                                     start=True, stop=True)
                    E = rpool.tile([128, CW], f32, tag=f"E{h}")
                    nc.vector.scalar_tensor_tensor(
                        E[:], ps[:], c, NF[:, sl], AL.mult, AL.max)
                    use_pool = (D_ENGINE == "pool"
                                or (D_ENGINE == "split" and h % 2 == 0))
                    if i < iters:
                        if use_pool:
                            # Pool can only do plain tensor_tensor: stage
                            # T = g_{i+1}*E on ACT (off critical path).
                            T = wpool.tile([128, CW], f32, tag=f"T{h}")
                            nc.scalar.activation(T[:], E[:], Copy, scale=gnext)
                        if i == 1:
                            Dh[h] = E        # D~_1 = E_1 (g_1 = 0)
                        elif use_pool:
                            D = vpool.tile([128, CW], f32, tag=f"D{h}")
                            nc.gpsimd.tensor_tensor(
                                D[:], E[:], Tprev[h][:], AL.subtract)
                            Dh[h] = D
                        else:
                            # single DVE stt: D~ = E - g_i * E_prev
                            D = vpool.tile([128, CW], f32, tag=f"D{h}")
                            nc.vector.scalar_tensor_tensor(
                                D[:], Eprev[h][:], -g, E[:], AL.mult, AL.add)
                            Dh[h] = D
                        if use_pool:
                            Tprev[h] = T
                    Eprev[h] = E
                    Eh[h] = E
            spool = bpool
            for h in range(CHUNKS):
                sl = slices[h]
                S = spool.tile([128, CW], f32, tag=f"S{h}", name=f"Sfin{h}")
                nc.vector.tensor_tensor(S[:], Eh[h][:], NF[:, sl], AL.subtract)
                nc.sync.dma_start(out=Sout_d[:, sl], in_=S[:])

    nc.compile()
    return nc


_build_program.consts = None


def _pack_inputs(X: np.ndarray, A: np.ndarray):
    f32 = np.float32
    step, H, Mw, Minv, cs, gs = _host_constants(A)
    Hd = np.zeros((128, 128), dtype=f32)
    for gi in range(4):
        Hd[32 * gi:32 * gi + 32, 32 * gi:32 * gi + 32] = H

    def pack(mat_t):                                  # (1024, 32) -> (128, 256)
        p = mat_t.reshape(8, 128, K_DIM).transpose(1, 0, 2)
        return np.ascontiguousarray(p.reshape(128, 8 * K_DIM))

    Apk = pack((A.astype(f32) * step).astype(f32))
    Mpk = pack(np.ascontiguousarray(Mw.T))
    Npk = pack(np.ascontiguousarray(Minv.T))
    in_maps = []
    for cidx in range(N_CORES):
        xs = X[:, N_SHARD * cidx:N_SHARD * (cidx + 1)].astype(f32)
        xs = np.ascontiguousarray(xs.reshape(8, 128, N_SHARD))
        in_maps.append({
            "Hd": Hd, "Apk": Apk, "Mpk": Mpk, "Npk": Npk, "Xs": xs,
        })
    return in_maps, (cs, gs)


def _unstack(results):
    out = np.empty((K_DIM, N_DIM), dtype=np.float32)
    for cidx in range(N_CORES):
        st = results[cidx]["Sout"]            # (128, N_STACK)
        blk = st.reshape(4, 32, N_STACK).transpose(1, 0, 2).reshape(32, N_SHARD)
        out[:, N_SHARD * cidx:N_SHARD * (cidx + 1)] = blk
    return out


def kernel(X: np.ndarray, A: np.ndarray) -> np.ndarray:
    from concourse import bass_utils

    in_maps, consts = _pack_inputs(np.asarray(X), np.asarray(A))
    _build_program.consts = consts
    nc = _build_program(ITERS)
    res = bass_utils.run_bass_kernel_spmd(
        nc, in_maps, list(range(N_CORES)),
        trace=bool(int(os.environ.get("NNLS_KERNEL_TRACE", "0"))),
    )
    kernel.last_results = res
    kernel.last_nc = nc
    kernel.last_in_maps = in_maps
    return _unstack(res.results)


kernel.last_results = None


# revision 17
# speedup vs baseline: 1.5131x; 1.5131x over previous
"""Trainium2 Bass kernel for batched NNLS via accelerated projected gradient.

Strategy (hardcoded for M=1024, K=32, N=16384, 8 cores):
  - Shard the n (column) dimension of X across 8 cores (2048 cols/core).
  - Host precomputes tiny A-only quantities exactly as the fp32 reference
    does where shared (G = A^T A, power iteration for L, step = 1/L,
    H = I - step*G), plus a ridge-regularized warm-start map
    Mw = (G + delta*I)^{-1} A^T and a tuned momentum schedule
    (t-rule ramp for the first RAMP iters, then constant BETA).
  - Device per core, phase 1: B = (step*A)^T X_shard and S0 = relu(Mw X_shard)
    via accumulating fp32 matmuls (k=32 packed 4x along partitions so every
    op runs 128 partitions wide; free dim is the 512 stacked columns).
  - Iteration i = 1..ITERS (same fixed point as the reference's FISTA, so it
    converges to the same solution; the tuned schedule + warm start reach the
    2e-2 gate with ~10x margin in 48 iterations instead of 500):
      ps = H' @ V            (PE, fp32, fixed block-diag weights)
      W  = c_{i-1}*ps + B    (DVE scalar_tensor_tensor, PSUM read)
      R  = relu(W)           (ACT)
      V' = R - g_i*R_prev    (Pool/GPSIMD scalar_tensor_tensor, SBUF only)
    with V = Y/c the momentum iterate rescaled so per-iteration scalars ride
    in instruction immediates.  Four engines pipeline across 2 column halves.
  - Output S = R_final directly (it is already relu(W_final)).
"""

import os
import sys

import numpy as np

sys.path.insert(0, "/opt/trn_rl_repo")

M_DIM, K_DIM, N_DIM = 1024, 32, 16384
N_CORES = 8
N_SHARD = N_DIM // N_CORES          # 2048
N_STACK = N_SHARD // 4              # 512 cols in 4-group stacked layout
ITERS = int(os.environ.get("NNLS_KERNEL_ITERS", "44"))
HALF = N_STACK // 2                 # 256
DELTA = 3.0                         # warm-start ridge
RAMP = 8                            # t-rule momentum ramp length
BETA = 0.86                         # constant momentum after ramp
V_ENGINE = os.environ.get("NNLS_V_ENGINE", "dve")  # dve | pool | split
SCHEME = os.environ.get("NNLS_SCHEME", "shift")    # shift | base
D_ENGINE = os.environ.get("NNLS_D_ENGINE", "dve")   # dve | pool | split
CHUNKS = int(os.environ.get("NNLS_CHUNKS", "2"))    # independent col pipelines


def _host_constants(A: np.ndarray):
    """fp32 replication of the reference's G/L/step plus warm start + schedule."""
    f32 = np.float32
    A = A.astype(f32)
    G = (A.T @ A).astype(f32)
    v = np.ones(K_DIM, dtype=f32) / f32(np.sqrt(f32(K_DIM)))
    for _ in range(100):
        w = (G @ v).astype(f32)
        v = (w / (f32(np.linalg.norm(w)) + f32(1e-30))).astype(f32)
    L = f32(v @ (G @ v))
    step = f32(1.0) / L
    H = (np.eye(K_DIM, dtype=f32) - step * G).astype(f32)

    Mw = np.linalg.solve(
        (G + f32(DELTA) * np.eye(K_DIM, dtype=f32)).astype(np.float64),
        A.T.astype(np.float64),
    ).astype(f32)                            # (k, m) warm-start map
    Minv = np.linalg.solve(
        G.astype(np.float64), A.T.astype(np.float64)
    ).astype(f32)                            # (k, m): W_fix = Minv @ X

    t = 1.0
    mus = []
    for j in range(1024):
        tn = 0.5 * (1.0 + np.sqrt(1.0 + 4.0 * t * t))
        mu = (t - 1.0) / tn
        t = tn
        mus.append(min(mu, BETA) if j < RAMP else BETA)
    mus = np.array(mus, dtype=f32)           # mus[j] = mu_{j+1}
    cs = (1.0 + mus).astype(f32)             # cs[j] = 1 + mu_{j+1}
    gs = (mus / (1.0 + mus)).astype(f32)     # gs[j] = g_{j+1}
    return step, H, Mw, Minv, cs, gs


def _build_program(iters: int):
    import concourse.bass as bass
    import concourse.tile as tile
    from concourse import bacc, mybir
    from contextlib import ExitStack

    f32 = mybir.dt.float32
    AL = mybir.AluOpType
    nc = bacc.Bacc("TRN2", target_bir_lowering=False, debug=False,
                   num_devices=N_CORES)

    Hd_d = nc.dram_tensor("Hd", [128, 128], f32, kind="ExternalInput").ap()
    if SCHEME == "base":
        Apk_d = nc.dram_tensor("Apk", [128, 8 * K_DIM], f32,
                               kind="ExternalInput").ap()
    else:
        Npk_d = nc.dram_tensor("Npk", [128, 8 * K_DIM], f32,
                               kind="ExternalInput").ap()
    Mpk_d = nc.dram_tensor("Mpk", [128, 8 * K_DIM], f32, kind="ExternalInput").ap()
    Xs_d = nc.dram_tensor("Xs", [8, 128, N_SHARD], f32, kind="ExternalInput").ap()
    Sout_d = nc.dram_tensor("Sout", [128, N_STACK], f32, kind="ExternalOutput").ap()

    cs_arr, gs_arr = _build_program.consts
    Relu = mybir.ActivationFunctionType.Relu
    Copy = mybir.ActivationFunctionType.Copy

    with tile.TileContext(nc) as tc, ExitStack() as ctx:
        const_pool = ctx.enter_context(tc.tile_pool(name="const", bufs=1))
        xpool = ctx.enter_context(tc.tile_pool(name="x", bufs=1))
        bpool = ctx.enter_context(tc.tile_pool(name="b", bufs=1))
        wpool = ctx.enter_context(tc.tile_pool(name="w", bufs=3))
        rpool = ctx.enter_context(tc.tile_pool(name="r", bufs=3))
        vpool = ctx.enter_context(tc.tile_pool(name="v", bufs=2))
        psA_pool = ctx.enter_context(tc.tile_pool(name="psA", bufs=1, space="PSUM"))
        psW_pool = ctx.enter_context(tc.tile_pool(name="psW", bufs=1, space="PSUM"))
        ps_pool = ctx.enter_context(tc.tile_pool(name="ps", bufs=2, space="PSUM"))
        scr_pool = ctx.enter_context(tc.tile_pool(name="scr", bufs=1, space="PSUM"))

        Hd = const_pool.tile([128, 128], f32, tag="Hd")
        nc.sync.dma_start(out=Hd[:], in_=Hd_d[:])
        if SCHEME == "base":
            Apk = const_pool.tile([128, 8 * K_DIM], f32, tag="Apk")
            nc.sync.dma_start(out=Apk[:], in_=Apk_d[:])
        else:
            Npk = const_pool.tile([128, 8 * K_DIM], f32, tag="Npk")
            nc.sync.dma_start(out=Npk[:], in_=Npk_d[:])
        Mpk = const_pool.tile([128, 8 * K_DIM], f32, tag="Mpk")
        nc.sync.dma_start(out=Mpk[:], in_=Mpk_d[:])

        # LDW-bearing matmuls only support one sync-wait command; these dummy
        # matmuls fold the const DMA ticks into PE's vector clock so every
        # real matmul below needs at most one wait.
        consts2 = (Apk,) if SCHEME == "base" else (Npk,)
        for ci, ct in enumerate((Hd, Mpk) + consts2):
            scr = scr_pool.tile([128, 8], f32, tag="scr", name=f"scr{ci}")
            nc.tensor.matmul(scr[0:1, 0:1], ct[:, 0:1], ct[:, 0:1],
                             start=True, stop=True)

        # ---- Phase 1 reductions over X (4 column groups stacked on partitions)
        xts = []
        for j in range(8):
            xt = xpool.tile([128, N_SHARD], f32, tag=f"xt{j}", name=f"xt{j}")
            nc.sync.dma_start(out=xt[:], in_=Xs_d[j])
            xts.append(xt)

        def reduce_chain(weights, pstile):
            for grp in range(4):
                for j in range(8):
                    nc.tensor.matmul(
                        pstile[32 * grp:32 * grp + 32, :],
                        weights[:, 32 * j:32 * j + 32],
                        xts[j][:, N_STACK * grp:N_STACK * (grp + 1)],
                        start=(j == 0),
                        stop=(j == 7),
                        tile_position=(0, 32 * grp),
                    )

        psW = psW_pool.tile([128, N_STACK], f32, tag="psW")
        reduce_chain(Mpk, psW)                   # Mw @ X (ridge warm start)
        S0 = bpool.tile([128, N_STACK], f32, tag="S0")
        nc.scalar.activation(S0[:], psW[:], Relu)

        if SCHEME == "base":
            psB = psA_pool.tile([128, N_STACK], f32, tag="psB")
            reduce_chain(Apk, psB)               # (step*A)^T X
            B = bpool.tile([128, N_STACK], f32, tag="B")
            nc.vector.tensor_scalar_add(B[:], psB[:], 0.0)

            Vh = [S0[:, 0:HALF], S0[:, HALF:N_STACK]]
            Rprev = [S0[:, 0:HALF], S0[:, HALF:N_STACK]]
            for i in range(1, iters + 1):
                c = 1.0 if i == 1 else float(cs_arr[i - 2])   # c_{i-1}
                g = float(gs_arr[i - 1])                      # g_i
                for h in range(2):
                    sl = slice(HALF * h, HALF * (h + 1))
                    ps = ps_pool.tile([128, HALF], f32, tag=f"ps{h}")
                    nc.tensor.matmul(ps[:], Hd[:], Vh[h][:],
                                     start=True, stop=True)
                    W = wpool.tile([128, HALF], f32, tag=f"W{h}")
                    nc.vector.scalar_tensor_tensor(
                        W[:], ps[:], c, B[:, sl], AL.mult, AL.add)
                    R = rpool.tile([128, HALF], f32, tag=f"R{h}")
                    nc.scalar.activation(R[:], W[:], Relu)
                    if i < iters:
                        V = vpool.tile([128, HALF], f32, tag=f"V{h}")
                        veng = (nc.gpsimd if V_ENGINE == "pool"
                                or (V_ENGINE == "split" and h == 0)
                                else nc.vector)
                        veng.scalar_tensor_tensor(
                            V[:], Rprev[h][:], -g, R[:], AL.mult, AL.add)
                        Vh[h] = V
                    Rprev[h] = R
            for h in range(2):
                sl = slice(HALF * h, HALF * (h + 1))
                nc.sync.dma_start(out=Sout_d[:, sl], in_=Rprev[h][:])
        else:
            # Shifted iteration: E = S - Wfix, NF = -Wfix.
            #   ps = H @ D~       (PE)
            #   E' = max(c*ps,NF) (DVE stt, PSUM read)
            #   T' = g_{i+1}*E'   (ACT copy-scale; consumed next iteration)
            #   D~' = E' - T      (Pool tensor_tensor)
            # Output S = E_final - NF.
            psN = psA_pool.tile([128, N_STACK], f32, tag="psN")
            reduce_chain(Npk, psN)               # Wfix = G^-1 A^T X
            NF = bpool.tile([128, N_STACK], f32, tag="NF")
            nc.vector.tensor_scalar_mul(NF[:], psN[:], -1.0)
            E0 = bpool.tile([128, N_STACK], f32, tag="E0")
            nc.vector.tensor_tensor(E0[:], S0[:], NF[:], AL.add)

            CW = N_STACK // CHUNKS               # columns per chunk
            ps_bufs = 2 if CHUNKS <= 2 else 1
            slices = [slice(CW * h, CW * (h + 1)) for h in range(CHUNKS)]
            Dh = [E0[:, sl] for sl in slices]
            Tprev = [None] * CHUNKS
            Eprev = [E0[:, sl] for sl in slices]
            Eh = [None] * CHUNKS
            for i in range(1, iters + 1):
                c = 1.0 if i == 1 else float(cs_arr[i - 2])   # c_{i-1}
                gnext = float(gs_arr[i])                      # g_{i+1}
                g = float(gs_arr[i - 1])                      # g_i
                for h in range(CHUNKS):
                    sl = slices[h]
                    ps = ps_pool.tile([128, CW], f32, tag=f"ps{h}",
                                      bufs=ps_bufs)
                    nc.tensor.matmul(ps[:], Hd[:], Dh[h][:],
                                     start=True, stop=True)
                    E = rpool.tile([128, CW], f32, tag=f"E{h}")
                    nc.vector.scalar_tensor_tensor(
                        E[:], ps[:], c, NF[:, sl], AL.mult, AL.max)
                    use_pool = (D_ENGINE == "pool"
                                or (D_ENGINE == "split" and h % 2 == 0))
                    if i < iters:
                        if use_pool:
                            # Pool can only do plain tensor_tensor: stage
                            # T = g_{i+1}*E on ACT (off critical path).
                            T = wpool.tile([128, CW], f32, tag=f"T{h}")
                            nc.scalar.activation(T[:], E[:], Copy, scale=gnext)
                        if i == 1:
                            Dh[h] = E        # D~_1 = E_1 (g_1 = 0)
                        elif use_pool:
                            D = vpool.tile([128, CW], f32, tag=f"D{h}")
                            nc.gpsimd.tensor_tensor(
                                D[:], E[:], Tprev[h][:], AL.subtract)
                            Dh[h] = D
                        else:
                            # single DVE stt: D~ = E - g_i * E_prev
                            D = vpool.tile([128, CW], f32, tag=f"D{h}")
                            nc.vector.scalar_tensor_tensor(
                                D[:], Eprev[h][:], -g, E[:], AL.mult, AL.add)
                            Dh[h] = D
                        if use_pool:
                            Tprev[h] = T
                    Eprev[h] = E
                    Eh[h] = E
            spool = bpool
            for h in range(CHUNKS):
                sl = slices[h]
                S = spool.tile([128, CW], f32, tag=f"S{h}", name=f"Sfin{h}")
                nc.vector.tensor_tensor(S[:], Eh[h][:], NF[:, sl], AL.subtract)
                nc.sync.dma_start(out=Sout_d[:, sl], in_=S[:])

    nc.compile()
    return nc


_build_program.consts = None


def _pack_inputs(X: np.ndarray, A: np.ndarray):
    f32 = np.float32
    step, H, Mw, Minv, cs, gs = _host_constants(A)
    Hd = np.zeros((128, 128), dtype=f32)
    for gi in range(4):
        Hd[32 * gi:32 * gi + 32, 32 * gi:32 * gi + 32] = H

    def pack(mat_t):                                  # (1024, 32) -> (128, 256)
        p = mat_t.reshape(8, 128, K_DIM).transpose(1, 0, 2)
        return np.ascontiguousarray(p.reshape(128, 8 * K_DIM))

    Apk = pack((A.astype(f32) * step).astype(f32))
    Mpk = pack(np.ascontiguousarray(Mw.T))
    Npk = pack(np.ascontiguousarray(Minv.T))
    in_maps = []
    for cidx in range(N_CORES):
        xs = X[:, N_SHARD * cidx:N_SHARD * (cidx + 1)].astype(f32)
        xs = np.ascontiguousarray(xs.reshape(8, 128, N_SHARD))
        in_maps.append({
            "Hd": Hd, "Apk": Apk, "Mpk": Mpk, "Npk": Npk, "Xs": xs,
        })
    return in_maps, (cs, gs)


def _unstack(results):
    out = np.empty((K_DIM, N_DIM), dtype=np.float32)
    for cidx in range(N_CORES):
        st = results[cidx]["Sout"]            # (128, N_STACK)
        blk = st.reshape(4, 32, N_STACK).transpose(1, 0, 2).reshape(32, N_SHARD)
        out[:, N_SHARD * cidx:N_SHARD * (cidx + 1)] = blk
    return out


def kernel(X: np.ndarray, A: np.ndarray) -> np.ndarray:
    from concourse import bass_utils

    in_maps, consts = _pack_inputs(np.asarray(X), np.asarray(A))
    _build_program.consts = consts
    nc = _build_program(ITERS)
    res = bass_utils.run_bass_kernel_spmd(
        nc, in_maps, list(range(N_CORES)),
        trace=bool(int(os.environ.get("NNLS_KERNEL_TRACE", "0"))),
    )
    kernel.last_results = res
    kernel.last_nc = nc
    kernel.last_in_maps = in_maps
    return _unstack(res.results)


kernel.last_results = None
